# revision 17
# baseline (speedup 1.0000x reference)
"""Causal multi-head attention on 8 trn2 NeuronCores.

Sharding: core c handles batch b=c//4 and heads [4*(c%4), 4*(c%4)+4).
Each core computes its 4 heads' attention plus the partial output
projection against the matching 256 rows of Wo; the host sums the 4
partials per batch (the all-reduce implied by row-sharding Wo) and adds
bo.

v7 (on top of v5's bf16 matmuls / fused biases / denominator-column
softmax / lagged ctx pipeline):
  - Input DMA rebuilt: weights packed host-side so each tensor lands in
    one large HW-DGE transfer (2-4KB per partition line); xt chunks
    0-3 stream on the scalar HW queue while chunks 4-7 ride the sync
    queue interleaved with the weights in demand order (the v5 software
    SWDGE queue delivered xt at ~260GB/s and starved the projection
    phase for ~14us); tiny consts ride the now-free gpsimd SW queue.
  - Q/K projections run c-outer/sb-inner (8 PSUM banks: 4 Q + 4 K per
    pair) so the PE consumes xt chunk-by-chunk right behind the DMA.
  - Causal mask applied as a 0/1 triangle multiply on DVE over the one
    128x128 diagonal block after exp (only that block is ever masked).
    Replaces v5's 64 identity-matmul -1e9 adds: ~20k PE columns plus
    their LDWEIGHTS leave the PE, the denominator stays exact because
    masking precedes the ones-column ctx matmul.
  - Score chunks processed in PAIRS sharing one 2-bank PSUM tile so a
    single ACT instruction exponentiates both (the ~340ns fixed cost
    per ACTIVATE made ACT the attention-phase pipeline limiter at one
    exp per chunk). Diag pairs exp a small garbage gap between their
    staircase regions; those pt columns are never read.
  - Output written fp16 (halves the 8MB/core store; fp16 beats bf16 on
    precision at this scale), one [128,1024] DMA per row block.
"""

import sys

for _p in ("/opt/trn_rl_repo", "/root/.axon_site/_ro/trn_rl_repo"):
    if _p not in sys.path:
        sys.path.insert(0, _p)

import numpy as np

import concourse.bass as bass
import concourse.bacc as bacc
import concourse.tile as tile
from concourse import mybir
from concourse.bass_utils import run_bass_kernel_spmd

F32 = mybir.dt.float32
F16 = mybir.dt.float16
BF16 = mybir.dt.bfloat16

B, S, D, H, DK = 2, 2048, 1024, 16, 64
NCORES = 8
HPC = 4          # heads per core
NPAIR = 2        # head pairs per core
ND = D // 128    # 8 contraction chunks over d
NS = S // 512    # 4 query blocks
NS16 = S // 128  # 16 sequence chunks

_CACHE = {}


def _build_bass():
    nc = bacc.Bacc(None)
    xt = nc.dram_tensor("xt", [128, ND, S], BF16, kind="ExternalInput")
    wq = nc.dram_tensor("wq", [128, NPAIR, ND, 128], BF16, kind="ExternalInput")
    wk = nc.dram_tensor("wk", [128, NPAIR, ND, 128], BF16, kind="ExternalInput")
    wv = nc.dram_tensor("wv", [128, ND, 256], BF16, kind="ExternalInput")
    wo = nc.dram_tensor("wo", [128, 2, D], BF16, kind="ExternalInput")
    bq = nc.dram_tensor("bq", [128, NPAIR], F32, kind="ExternalInput")
    bv_bc = nc.dram_tensor("bv_bc", [128, NPAIR, 2, 64], F32, kind="ExternalInput")
    mask01 = nc.dram_tensor("mask01", [128, 128], BF16, kind="ExternalInput")
    sel = nc.dram_tensor("sel", [2, 128], BF16, kind="ExternalInput")
    vfix = nc.dram_tensor("vfix", [128, 64], BF16, kind="ExternalInput")
    out = nc.dram_tensor("out", [S, D], F16, kind="ExternalOutput")

    with nc.allow_low_precision("bf16 operands; accumulation stays fp32 in PSUM"), \
            tile.TileContext(nc) as tc:
        with (
            tc.tile_pool(name="consts", bufs=1) as consts,
            tc.tile_pool(name="qkv", bufs=1) as qkv,
        ):
            wq_sb = consts.tile([128, NPAIR, ND, 128], BF16, tag="wq")
            wk_sb = consts.tile([128, NPAIR, ND, 128], BF16, tag="wk")
            wv_sb = consts.tile([128, ND, 256], BF16, tag="wv")
            wo_sb = consts.tile([128, 2, D], BF16, tag="wo")
            bq_sb = consts.tile([128, NPAIR], F32, tag="bq")
            bv_sb = consts.tile([128, NPAIR, 2, 64], F32, tag="bv")
            mask_sb = consts.tile([128, 128], BF16, tag="mask01")
            sel_sb = consts.tile([2, 128], BF16, tag="sel")

            qt_sb = qkv.tile([128, NPAIR, S], BF16, tag="qt")
            kt_sb = qkv.tile([128, NPAIR, S], BF16, tag="kt")
            # Vaug per pair: cols 0:64 V_even | 64 ones | 65:128 zeros
            # | 128:192 V_odd. Even lhsT = cols 0:65 -> ctx on parts
            # 0:64 (+denominator row 64); odd lhsT = cols 64:192 ->
            # denominator on part 0, ctx on parts 64:128.
            va_sb = qkv.tile([128, NPAIR, NS16, 192], BF16, tag="va")
            ctxcat_sb = qkv.tile([128, 2, S], BF16, tag="ctxcat")

            with (
                tc.tile_pool(name="xp", bufs=1) as xp,
                tc.tile_pool(name="mmp", bufs=8, space="PSUM") as mmp,
            ):
                xt_sb = xp.tile([128, ND, S], BF16, tag="xt")
                # xt per-chunk on the scalar HW-DGE queue, weights as one
                # large DMA each on the sync HW queue, tiny consts on the
                # gpsimd SW queue: three queues stream in parallel and
                # every HW transfer moves 2-4KB per partition line.
                nc.scalar.dma_start(out=wq_sb[:, 0], in_=wq[:, 0])
                nc.sync.dma_start(out=wk_sb[:, 0], in_=wk[:, 0])
                for c in range(ND):
                    eng = nc.scalar if c % 2 == 0 else nc.sync
                    eng.dma_start(out=xt_sb[:, c, :], in_=xt[:, c, :])
                nc.gpsimd.dma_start(out=bq_sb[:], in_=bq[:])
                nc.scalar.dma_start(out=wq_sb[:, 1], in_=wq[:, 1])
                nc.sync.dma_start(out=wk_sb[:, 1], in_=wk[:, 1])
                nc.sync.dma_start(out=wv_sb[:], in_=wv[:])
                nc.sync.dma_start(out=wo_sb[:], in_=wo[:])
                nc.gpsimd.dma_start(out=bv_sb[:], in_=bv_bc[:])
                nc.gpsimd.dma_start(out=mask_sb[:], in_=mask01[:])
                nc.gpsimd.dma_start(out=sel_sb[:], in_=sel[:])
                for p in range(NPAIR):
                    vfix_bc = bass.AP(
                        tensor=vfix.ap().tensor,
                        offset=0,
                        ap=[[64, 128], [0, NS16], [1, 64]],
                    )
                    nc.gpsimd.dma_start(out=va_sb[:, p, :, 64:128], in_=vfix_bc)

                # ---- Q^T / K^T projections (per pair, dk on partitions).
                # c-outer so the PE wants xt chunk c only ~1.7us after
                # chunk c-1: it trails right behind the streaming DMA.
                for p in range(NPAIR):
                    qps = [
                        mmp.tile([128, 512], F32, tag="mm", name=f"qp{sb}")
                        for sb in range(NS)
                    ]
                    kps = [
                        mmp.tile([128, 512], F32, tag="mm", name=f"kp{sb}")
                        for sb in range(NS)
                    ]
                    for c in range(ND):
                        for sb in range(NS):
                            nc.tensor.matmul(
                                qps[sb][:],
                                lhsT=wq_sb[:, p, c, :],
                                rhs=xt_sb[:, c, sb * 512:(sb + 1) * 512],
                                start=(c == 0),
                                stop=(c == ND - 1),
                            )
                        for sb in range(NS):
                            nc.tensor.matmul(
                                kps[sb][:],
                                lhsT=wk_sb[:, p, c, :],
                                rhs=xt_sb[:, c, sb * 512:(sb + 1) * 512],
                                start=(c == 0),
                                stop=(c == ND - 1),
                            )
                    for sb in range(NS):
                        nc.scalar.activation(
                            out=qt_sb[:, p, sb * 512:(sb + 1) * 512],
                            in_=qps[sb][:],
                            func=mybir.ActivationFunctionType.Identity,
                            bias=bq_sb[:, p:p + 1],
                            scale=1.0,
                        )
                        nc.vector.tensor_copy(
                            out=kt_sb[:, p, sb * 512:(sb + 1) * 512],
                            in_=kps[sb][:],
                        )

                # ---- V in natural layout [s, dk], 4 heads at once.
                # bv is added during the eviction (tensor_add with a
                # partition-broadcast constant): exact through the softmax
                # denominator trick since rows of P sum to den.
                for s16 in range(NS16):
                    vp = mmp.tile([128, 256], F32, tag="mm", name="vp")
                    for c in range(ND):
                        nc.tensor.matmul(
                            vp[:],
                            lhsT=xt_sb[:, c, s16 * 128:(s16 + 1) * 128],
                            rhs=wv_sb[:, c, :],
                            start=(c == 0),
                            stop=(c == ND - 1),
                        )
                    # V_even -> va cols 0:64, V_odd -> cols 128:192 in one
                    # two-segment add per pair
                    for p in range(NPAIR):
                        d0 = va_sb[:, p, s16, 0:64]
                        dst = bass.AP(
                            tensor=d0.tensor, offset=d0.offset,
                            ap=[[d0.ap[0][0], 128], [128, 2], [1, 64]],
                        )
                        s0 = vp[:, p * 128:(p + 1) * 128]
                        src = bass.AP(
                            tensor=s0.tensor, offset=s0.offset,
                            ap=[[s0.ap[0][0], 128], [64, 2], [1, 64]],
                        )
                        nc.vector.tensor_add(
                            out=dst, in0=src, in1=bv_sb[:, p, :, :]
                        )

            # ---- attention + output projection, per query block
            with (
                tc.tile_pool(name="stp", bufs=2, space="PSUM") as stp,
                tc.tile_pool(name="ctxp", bufs=2, space="PSUM") as ctxp,
                tc.tile_pool(name="ptp", bufs=5) as ptp,
                tc.tile_pool(name="smp", bufs=3) as smp,
                tc.tile_pool(name="outp", bufs=3) as outp,
            ):
                def emit_norm_pair(ctx_e, ctx_o, den2, p, qb):
                    # paired normalization: one sel-matmul broadcasts BOTH
                    # heads' denominators (even -> partitions 0:64, odd ->
                    # 64:128, matching the ctx parity layout), one
                    # reciprocal serves both. Emitted lagged into the next
                    # head's score stream so the PE has work in flight
                    # while DVE turns the denominators into reciprocals.
                    # custom-DVE ops (and tile_position=(0,64) matmuls)
                    # misbehave on HW when based at partition 64, so
                    # everything stays at base 0.
                    bc_ps = ctxp.tile([128, 512], F32, tag="op", name="bc_ps", bufs=2)
                    nc.tensor.matmul(
                        bc_ps[:],
                        lhsT=sel_sb[:],
                        rhs=den2[:],
                        start=True,
                        stop=True,
                    )
                    rcp = smp.tile([128, 512], F32, tag="rcp", name="rcp")
                    nc.vector.reciprocal_approx_fast(
                        out=rcp[:], in_=bc_ps[:]
                    )
                    nc.vector.tensor_mul(
                        out=ctxcat_sb[0:64, p, qb * 512:(qb + 1) * 512],
                        in0=ctx_e[0:64, :],
                        in1=rcp[0:64, :],
                    )
                    nc.vector.tensor_mul(
                        out=ctxcat_sb[64:128, p, qb * 512:(qb + 1) * 512],
                        in0=ctx_o[64:128, :],
                        in1=rcp[64:128, :],
                    )

                def emit_outproj(qb):
                    for s16 in range(qb * 4, (qb + 1) * 4):
                        ot = outp.tile([128, D], F16, tag="ot", name="ot")
                        for do in range(2):
                            op = ctxp.tile([128, 512], F32, tag="op", name="op", bufs=2)
                            nc.tensor.matmul(
                                op[:],
                                lhsT=ctxcat_sb[:, 0, s16 * 128:(s16 + 1) * 128],
                                rhs=wo_sb[:, 0, do * 512:(do + 1) * 512],
                                start=True,
                                stop=False,
                            )
                            nc.tensor.matmul(
                                op[:],
                                lhsT=ctxcat_sb[:, 1, s16 * 128:(s16 + 1) * 128],
                                rhs=wo_sb[:, 1, do * 512:(do + 1) * 512],
                                start=False,
                                stop=True,
                            )
                            if do == 0:
                                nc.scalar.copy(
                                    out=ot[:, 0:512], in_=op[:]
                                )
                            else:
                                nc.vector.tensor_copy(
                                    out=ot[:, 512:1024], in_=op[:]
                                )
                        nc.sync.dma_start(
                            out=out[s16 * 128:(s16 + 1) * 128, :],
                            in_=ot[:],
                        )

                pending = None  # (ctx_e, ctx_o, den2, p, qb) awaiting norm
                cur = None      # (ctx_e, den2) of the in-flight even head
                for qb in range(NS):
                    nch = (qb + 1) * 4
                    for h in range(HPC):
                        p, j = h // 2, h % 2
                        even = j == 0
                        qs = qt_sb[j * 64:(j + 1) * 64, p, qb * 512:(qb + 1) * 512]
                        ctx_ps = ctxp.tile([128, 512], F32, tag="ctx", name="ctx_ps")
                        ctx_out = ctx_ps[0:65, :] if even else ctx_ps[:]
                        lagged = []  # (pt, f0, diag, c) awaiting their ctx mms

                        def emit_ctx(lag):
                            pt, f0, diag, c = lag
                            lhsT_v = (
                                va_sb[:, p, c, 0:65]
                                if even
                                else va_sb[:, p, c, 64:192]
                            )
                            nc.tensor.matmul(
                                ctx_out[:, f0:512] if diag else ctx_out,
                                lhsT=lhsT_v,
                                rhs=pt[:, f0:512],
                                start=(c == 0),
                                stop=(c == nch - 1),
                            )

                        # chunks in pairs sharing a 2-bank PSUM tile: two
                        # score matmuls, then ONE exp over both halves
                        # (ACT's ~340ns fixed cost per instruction made it
                        # the pipeline limiter at one exp per chunk).
                        for cp in range(nch // 2):
                            c0 = 2 * cp
                            st = stp.tile([128, 1024], F32, tag="st", name="st")
                            pt = ptp.tile([128, 1024], BF16, tag="pt", name="pt")
                            diag = c0 >= qb * 4
                            # columns [0, f0) of a diag block are fully
                            # masked (q < kv everywhere): skip them.
                            f00 = 128 * (c0 - qb * 4) if diag else 0
                            for ci in range(2):
                                c = c0 + ci
                                f0 = f00 + 128 * ci if diag else 0
                                nc.tensor.matmul(
                                    st[:, ci * 512 + f0:(ci + 1) * 512],
                                    lhsT=kt_sb[j * 64:(j + 1) * 64, p,
                                               c * 128:(c + 1) * 128],
                                    rhs=qs[:, f0:512],
                                    start=True,
                                    stop=True,
                                )
                            # one exp across both banks; for diag pairs the
                            # staircase leaves a gap of unwritten columns in
                            # the middle - exp'd garbage no one ever reads.
                            nc.scalar.activation(
                                out=pt[:, f00:1024],
                                in_=st[:, f00:1024],
                                func=mybir.ActivationFunctionType.Exp,
                            )
                            if diag:
                                # zero the two masked 128-wide triangle
                                # blocks (at f00 and 640+f00) in one DVE op;
                                # cheaper than the PE -1e9 add and keeps the
                                # denominator (ones-column matmul) exact.
                                blk = pt[:, f00:f00 + 128]
                                dst = bass.AP(
                                    tensor=blk.tensor, offset=blk.offset,
                                    ap=[[blk.ap[0][0], 128], [640, 2], [1, 128]],
                                )
                                msk = bass.AP(
                                    tensor=mask_sb.tensor, offset=mask_sb.offset,
                                    ap=[[mask_sb.ap[0][0], 128], [0, 2], [1, 128]],
                                )
                                nc.vector.tensor_mul(
                                    out=dst, in0=dst, in1=msk
                                )
                            # ctx trails two pairs behind its exp so the PE
                            # queue stays deep (hides ACT latency, semaphore
                            # propagation, and LDWEIGHTS of the next matmul).
                            for ci in range(2):
                                f0 = f00 + 128 * ci if diag else 0
                                lagged.append(
                                    (pt[:, ci * 512:(ci + 1) * 512],
                                     f0, diag, c0 + ci)
                                )
                            while len(lagged) > 4:
                                emit_ctx(lagged.pop(0))
                            if cp == 1:
                                # previous pair's norm + previous qb's out
                                # projection slot in here, after this head's
                                # first score pair is already in the PE queue
                                if pending is not None:
                                    emit_norm_pair(*pending)
                                    pending = None
                                if h == 0 and qb > 0:
                                    emit_outproj(qb - 1)
                        for lag in lagged:
                            emit_ctx(lag)

                        # denominator rows -> den2. DVE partition bases
                        # must be 32-aligned, so: copy the odd head's den
                        # (duplicated on ctx partitions 0 AND 1 by the
                        # second ones-column) as [0:2], then overwrite
                        # partition 0 with the even head's den (row 64).
                        if even:
                            cur = ctx_ps
                        else:
                            ctx_e = cur
                            den2 = smp.tile([2, 512], BF16, tag="den", name="den2")
                            nc.vector.tensor_copy(
                                out=den2[0:2, :], in_=ctx_ps[0:2, :]
                            )
                            nc.vector.tensor_copy(
                                out=den2[0:1, :], in_=ctx_e[64:65, :]
                            )
                            pending = (ctx_e, ctx_ps, den2, p, qb)

                emit_norm_pair(*pending)
                pending = None
                emit_outproj(NS - 1)
    if not nc.is_finalized():
        nc.finalize()
    return nc


def _prep_inputs(embeddings, Wq, bq, Wk, bk, Wv, bv, Wo, bo):
    embeddings = np.asarray(embeddings, np.float32)
    Wq, bq = np.asarray(Wq, np.float32), np.asarray(bq, np.float32)
    Wk = np.asarray(Wk, np.float32)
    Wv, bv = np.asarray(Wv, np.float32), np.asarray(bv, np.float32)
    Wo = np.asarray(Wo, np.float32)

    import ml_dtypes
    bf16_t = ml_dtypes.bfloat16
    # mask01[p, j] = 1 where col j (query) >= partition p (key) in the
    # 128x128 diagonal block, else 0
    mask01 = np.triu(np.ones((128, 128), np.float32)).astype(bf16_t)
    vfix = np.zeros((128, 64), np.float32)
    # two ones columns: va col 64 puts the denominator on partition 64 for
    # even heads / partition 0 for odd heads; va col 65 duplicates it on
    # partition 1 so the odd-head den copy needs no partition shift
    vfix[:, 0] = 1.0
    vfix[:, 1] = 1.0
    vfix = vfix.astype(bf16_t)
    sel = np.zeros((2, 128), np.float32)
    sel[0, 0:64] = 1.0
    sel[1, 64:128] = 1.0
    sel = sel.astype(bf16_t)

    in_maps = []
    for c in range(NCORES):
        b, g = c // 4, c % 4
        hs = HPC * g
        # [128, ND, S]: partition = d % 128, chunk = d // 128
        xt = np.ascontiguousarray(
            embeddings[b].T.reshape(ND, 128, S).transpose(1, 0, 2)
        ).astype(bf16_t)
        # 1/sqrt(dk) folded into Wq/bq (exact power of two)
        wq2 = np.stack(
            [np.concatenate([Wq[hs + 2 * p], Wq[hs + 2 * p + 1]], axis=1)
             for p in range(NPAIR)]
        ) * 0.125
        wk2 = np.stack(
            [np.concatenate([Wk[hs + 2 * p], Wk[hs + 2 * p + 1]], axis=1)
             for p in range(NPAIR)]
        )
        # [NPAIR, D, 128] -> [128, NPAIR, ND, 128] single-DMA layout
        wq2 = wq2.reshape(NPAIR, ND, 128, 128).transpose(2, 0, 1, 3)
        wk2 = wk2.reshape(NPAIR, ND, 128, 128).transpose(2, 0, 1, 3)
        wv4 = np.concatenate([Wv[hs + h] for h in range(HPC)], axis=1)
        wv4 = wv4.reshape(ND, 128, 256).transpose(1, 0, 2)
        wo4 = Wo[hs * DK:(hs + HPC) * DK, :].reshape(2, 128, D).transpose(1, 0, 2)
        bq2 = np.stack(
            [np.concatenate([bq[hs + 2 * p], bq[hs + 2 * p + 1]]) / 8.0
             for p in range(NPAIR)], axis=1
        )
        bvb = np.zeros((128, NPAIR, 2, 64), np.float32)
        for p in range(NPAIR):
            bvb[:, p, 0, :] = bv[hs + 2 * p][None, :]
            bvb[:, p, 1, :] = bv[hs + 2 * p + 1][None, :]
        in_maps.append({
            "xt": xt,
            "wq": np.ascontiguousarray(wq2).astype(bf16_t),
            "wk": np.ascontiguousarray(wk2).astype(bf16_t),
            "wv": np.ascontiguousarray(wv4).astype(bf16_t),
            "wo": np.ascontiguousarray(wo4).astype(bf16_t),
            "bq": np.ascontiguousarray(bq2),
            "bv_bc": bvb,
            "mask01": mask01,
            "sel": sel,
            "vfix": vfix,
        })
    return in_maps


def kernel(embeddings, Wq, bq, Wk, bk, Wv, bv, Wo, bo, _trace=False, _trace_kw=None):
    if "nc" not in _CACHE:
        _CACHE["nc"] = _build_bass()
    nc = _CACHE["nc"]
    in_maps = _prep_inputs(embeddings, Wq, bq, Wk, bk, Wv, bv, Wo, bo)
    kw = dict(_trace_kw or {})
    res = run_bass_kernel_spmd(
        nc, in_maps, core_ids=list(range(NCORES)), trace=_trace, **kw
    )
    _CACHE["last_result"] = res
    bo32 = np.asarray(bo, np.float32)
    out = np.empty((B, S, D), np.float32)
    for b in range(B):
        acc = np.asarray(res.results[4 * b]["out"], np.float32).copy()
        for g in range(1, 4):
            acc += np.asarray(res.results[4 * b + g]["out"], np.float32)
        out[b] = acc + bo32
    return out


# revision 19
# speedup vs baseline: 1.0041x; 1.0041x over previous
"""Causal multi-head attention on 8 trn2 NeuronCores.

Sharding: core c handles batch b=c//4 and heads [4*(c%4), 4*(c%4)+4).
Each core computes its 4 heads' attention plus the partial output
projection against the matching 256 rows of Wo; the host sums the 4
partials per batch (the all-reduce implied by row-sharding Wo) and adds
bo.

v7 (on top of v5's bf16 matmuls / fused biases / denominator-column
softmax / lagged ctx pipeline):
  - Input DMA rebuilt: weights packed host-side so each tensor lands in
    one large HW-DGE transfer (2-4KB per partition line); xt chunks
    0-3 stream on the scalar HW queue while chunks 4-7 ride the sync
    queue interleaved with the weights in demand order (the v5 software
    SWDGE queue delivered xt at ~260GB/s and starved the projection
    phase for ~14us); tiny consts ride the now-free gpsimd SW queue.
  - Q/K projections run c-outer/sb-inner (8 PSUM banks: 4 Q + 4 K per
    pair) so the PE consumes xt chunk-by-chunk right behind the DMA.
  - Causal mask applied as a 0/1 triangle multiply on DVE over the one
    128x128 diagonal block after exp (only that block is ever masked).
    Replaces v5's 64 identity-matmul -1e9 adds: ~20k PE columns plus
    their LDWEIGHTS leave the PE, the denominator stays exact because
    masking precedes the ones-column ctx matmul.
  - Score chunks processed in PAIRS sharing one 2-bank PSUM tile so a
    single ACT instruction exponentiates both (the ~340ns fixed cost
    per ACTIVATE made ACT the attention-phase pipeline limiter at one
    exp per chunk). Diag pairs exp a small garbage gap between their
    staircase regions; those pt columns are never read.
  - Output written fp16 (halves the 8MB/core store; fp16 beats bf16 on
    precision at this scale), one [128,1024] DMA per row block.
"""

import sys

for _p in ("/opt/trn_rl_repo", "/root/.axon_site/_ro/trn_rl_repo"):
    if _p not in sys.path:
        sys.path.insert(0, _p)

import numpy as np

import concourse.bass as bass
import concourse.bacc as bacc
import concourse.tile as tile
from concourse import mybir
from concourse.bass_utils import run_bass_kernel_spmd

F32 = mybir.dt.float32
F16 = mybir.dt.float16
BF16 = mybir.dt.bfloat16

B, S, D, H, DK = 2, 2048, 1024, 16, 64
NCORES = 8
HPC = 4          # heads per core
NPAIR = 2        # head pairs per core
ND = D // 128    # 8 contraction chunks over d
NS = S // 512    # 4 query blocks
NS16 = S // 128  # 16 sequence chunks

_CACHE = {}


def _build_bass():
    nc = bacc.Bacc(None)
    xt = nc.dram_tensor("xt", [128, ND, S], BF16, kind="ExternalInput")
    wq = nc.dram_tensor("wq", [128, NPAIR, ND, 128], BF16, kind="ExternalInput")
    wk = nc.dram_tensor("wk", [128, NPAIR, ND, 128], BF16, kind="ExternalInput")
    wv = nc.dram_tensor("wv", [128, ND, 256], BF16, kind="ExternalInput")
    wo = nc.dram_tensor("wo", [128, 2, D], BF16, kind="ExternalInput")
    bq = nc.dram_tensor("bq", [128, NPAIR], F32, kind="ExternalInput")
    bv_bc = nc.dram_tensor("bv_bc", [128, NPAIR, 2, 64], F32, kind="ExternalInput")
    mask01 = nc.dram_tensor("mask01", [128, 128], BF16, kind="ExternalInput")
    ones = nc.dram_tensor("ones", [1, 128], BF16, kind="ExternalInput")
    vfix = nc.dram_tensor("vfix", [128, 64], BF16, kind="ExternalInput")
    out = nc.dram_tensor("out", [S, D], F16, kind="ExternalOutput")

    with nc.allow_low_precision("bf16 operands; accumulation stays fp32 in PSUM"), \
            tile.TileContext(nc) as tc:
        with (
            tc.tile_pool(name="consts", bufs=1) as consts,
            tc.tile_pool(name="qkv", bufs=1) as qkv,
        ):
            wq_sb = consts.tile([128, NPAIR, ND, 128], BF16, tag="wq")
            wk_sb = consts.tile([128, NPAIR, ND, 128], BF16, tag="wk")
            wv_sb = consts.tile([128, ND, 256], BF16, tag="wv")
            wo_sb = consts.tile([128, 2, D], BF16, tag="wo")
            bq_sb = consts.tile([128, NPAIR], F32, tag="bq")
            bv_sb = consts.tile([128, NPAIR, 2, 64], F32, tag="bv")
            mask_sb = consts.tile([128, 128], BF16, tag="mask01")
            ones_sb = consts.tile([1, 128], BF16, tag="ones")

            qt_sb = qkv.tile([128, NPAIR, S], BF16, tag="qt")
            kt_sb = qkv.tile([128, NPAIR, S], BF16, tag="kt")
            # Vaug per pair: cols 0:64 V_even | 64 ones | 65:128 zeros
            # | 128:192 V_odd. Even lhsT = cols 0:65 -> ctx on parts
            # 0:64 (+denominator row 64); odd lhsT = cols 64:192 ->
            # denominator on part 0, ctx on parts 64:128.
            va_sb = qkv.tile([128, NPAIR, NS16, 192], BF16, tag="va")
            ctxcat_sb = qkv.tile([128, 2, S], BF16, tag="ctxcat")

            with (
                tc.tile_pool(name="xp", bufs=1) as xp,
                tc.tile_pool(name="mmp", bufs=8, space="PSUM") as mmp,
            ):
                xt_sb = xp.tile([128, ND, S], BF16, tag="xt")
                # xt per-chunk on the scalar HW-DGE queue, weights as one
                # large DMA each on the sync HW queue, tiny consts on the
                # gpsimd SW queue: three queues stream in parallel and
                # every HW transfer moves 2-4KB per partition line.
                # demand-ordered across the two HW queues: the scalar
                # queue starts streaming ~2us before sync, so it carries
                # the pair-0 weights plus chunk 0 (what the first matmuls
                # block on); sync streams the remaining chunks, staying
                # one chunk ahead of the c-outer projection sweep.
                nc.scalar.dma_start(out=wq_sb[:, 0], in_=wq[:, 0])
                nc.scalar.dma_start(out=wk_sb[:, 0], in_=wk[:, 0])
                nc.scalar.dma_start(out=xt_sb[:, 0, :], in_=xt[:, 0, :])
                nc.gpsimd.dma_start(out=bq_sb[:], in_=bq[:])
                for c in range(1, ND):
                    nc.sync.dma_start(out=xt_sb[:, c, :], in_=xt[:, c, :])
                nc.scalar.dma_start(out=wq_sb[:, 1], in_=wq[:, 1])
                nc.scalar.dma_start(out=wk_sb[:, 1], in_=wk[:, 1])
                nc.sync.dma_start(out=wv_sb[:], in_=wv[:])
                nc.sync.dma_start(out=wo_sb[:], in_=wo[:])
                nc.gpsimd.dma_start(out=bv_sb[:], in_=bv_bc[:])
                nc.gpsimd.dma_start(out=mask_sb[:], in_=mask01[:])
                nc.gpsimd.dma_start(out=ones_sb[:], in_=ones[:])
                for p in range(NPAIR):
                    vfix_bc = bass.AP(
                        tensor=vfix.ap().tensor,
                        offset=0,
                        ap=[[64, 128], [0, NS16], [1, 64]],
                    )
                    nc.gpsimd.dma_start(out=va_sb[:, p, :, 64:128], in_=vfix_bc)

                # ---- Q^T / K^T projections (per pair, dk on partitions).
                # c-outer so the PE wants xt chunk c only ~1.7us after
                # chunk c-1: it trails right behind the streaming DMA.
                for p in range(NPAIR):
                    qps = [
                        mmp.tile([128, 512], F32, tag="mm", name=f"qp{sb}")
                        for sb in range(NS)
                    ]
                    kps = [
                        mmp.tile([128, 512], F32, tag="mm", name=f"kp{sb}")
                        for sb in range(NS)
                    ]
                    for c in range(ND):
                        for sb in range(NS):
                            nc.tensor.matmul(
                                qps[sb][:],
                                lhsT=wq_sb[:, p, c, :],
                                rhs=xt_sb[:, c, sb * 512:(sb + 1) * 512],
                                start=(c == 0),
                                stop=(c == ND - 1),
                            )
                        for sb in range(NS):
                            nc.tensor.matmul(
                                kps[sb][:],
                                lhsT=wk_sb[:, p, c, :],
                                rhs=xt_sb[:, c, sb * 512:(sb + 1) * 512],
                                start=(c == 0),
                                stop=(c == ND - 1),
                            )
                    for sb in range(NS):
                        nc.scalar.activation(
                            out=qt_sb[:, p, sb * 512:(sb + 1) * 512],
                            in_=qps[sb][:],
                            func=mybir.ActivationFunctionType.Identity,
                            bias=bq_sb[:, p:p + 1],
                            scale=1.0,
                        )
                        nc.vector.tensor_copy(
                            out=kt_sb[:, p, sb * 512:(sb + 1) * 512],
                            in_=kps[sb][:],
                        )

                # ---- V in natural layout [s, dk], 4 heads at once.
                # bv is added during the eviction (tensor_add with a
                # partition-broadcast constant): exact through the softmax
                # denominator trick since rows of P sum to den.
                for s16 in range(NS16):
                    vp = mmp.tile([128, 256], F32, tag="mm", name="vp")
                    for c in range(ND):
                        nc.tensor.matmul(
                            vp[:],
                            lhsT=xt_sb[:, c, s16 * 128:(s16 + 1) * 128],
                            rhs=wv_sb[:, c, :],
                            start=(c == 0),
                            stop=(c == ND - 1),
                        )
                    # V_even -> va cols 0:64, V_odd -> cols 128:192 in one
                    # two-segment add per pair
                    for p in range(NPAIR):
                        d0 = va_sb[:, p, s16, 0:64]
                        dst = bass.AP(
                            tensor=d0.tensor, offset=d0.offset,
                            ap=[[d0.ap[0][0], 128], [128, 2], [1, 64]],
                        )
                        s0 = vp[:, p * 128:(p + 1) * 128]
                        src = bass.AP(
                            tensor=s0.tensor, offset=s0.offset,
                            ap=[[s0.ap[0][0], 128], [64, 2], [1, 64]],
                        )
                        nc.vector.tensor_add(
                            out=dst, in0=src, in1=bv_sb[:, p, :, :]
                        )

            # ---- attention + output projection, per query block
            with (
                tc.tile_pool(name="stp", bufs=2, space="PSUM") as stp,
                tc.tile_pool(name="ctxp", bufs=2, space="PSUM") as ctxp,
                tc.tile_pool(name="ptp", bufs=5) as ptp,
                tc.tile_pool(name="smp", bufs=3) as smp,
                tc.tile_pool(name="outp", bufs=3) as outp,
            ):
                def emit_norm(ctx_ps, even, p, qb, h):
                    # normalization, partition-aligned per parity.
                    # Emitted one head late so the PE stream has score/ctx
                    # work in flight while DVE/PE turn the denominator
                    # into a broadcast reciprocal.
                    cs = 64 if even else 0
                    lo = 0 if even else 64
                    den = smp.tile([1, 512], BF16, tag="den", name="den")
                    nc.vector.tensor_copy(out=den[:], in_=ctx_ps[cs:cs + 1, :])
                    # broadcast to all 128 partitions: custom-DVE ops (and
                    # tile_position=(0,64) matmuls) misbehave on HW when
                    # based at partition 64, so keep everything at base 0.
                    bc_ps = ctxp.tile([128, 512], F32, tag="op", name="bc_ps", bufs=2)
                    nc.tensor.matmul(
                        bc_ps[:],
                        lhsT=ones_sb[0:1, :],
                        rhs=den[:],
                        start=True,
                        stop=True,
                    )
                    rcp = smp.tile([128, 512], F32, tag="rcp", name="rcp")
                    nc.vector.reciprocal_approx_fast(
                        out=rcp[:], in_=bc_ps[:]
                    )
                    nc.vector.tensor_mul(
                        out=ctxcat_sb[lo:lo + 64, p, qb * 512:(qb + 1) * 512],
                        in0=ctx_ps[lo:lo + 64, :],
                        in1=rcp[lo:lo + 64, :],
                    )

                def emit_outproj(qb):
                    for s16 in range(qb * 4, (qb + 1) * 4):
                        ot = outp.tile([128, D], F16, tag="ot", name="ot")
                        for do in range(2):
                            op = ctxp.tile([128, 512], F32, tag="op", name="op", bufs=2)
                            nc.tensor.matmul(
                                op[:],
                                lhsT=ctxcat_sb[:, 0, s16 * 128:(s16 + 1) * 128],
                                rhs=wo_sb[:, 0, do * 512:(do + 1) * 512],
                                start=True,
                                stop=False,
                            )
                            nc.tensor.matmul(
                                op[:],
                                lhsT=ctxcat_sb[:, 1, s16 * 128:(s16 + 1) * 128],
                                rhs=wo_sb[:, 1, do * 512:(do + 1) * 512],
                                start=False,
                                stop=True,
                            )
                            if do == 0:
                                nc.scalar.copy(
                                    out=ot[:, 0:512], in_=op[:]
                                )
                            else:
                                nc.vector.tensor_copy(
                                    out=ot[:, 512:1024], in_=op[:]
                                )
                        nc.sync.dma_start(
                            out=out[s16 * 128:(s16 + 1) * 128, :],
                            in_=ot[:],
                        )

                pending = None
                for qb in range(NS):
                    nch = (qb + 1) * 4
                    for h in range(HPC):
                        p, j = h // 2, h % 2
                        even = j == 0
                        qs = qt_sb[j * 64:(j + 1) * 64, p, qb * 512:(qb + 1) * 512]
                        ctx_ps = ctxp.tile([128, 512], F32, tag="ctx", name="ctx_ps")
                        ctx_out = ctx_ps[0:65, :] if even else ctx_ps[:]
                        lagged = []  # (pt, f0, diag, c) awaiting their ctx mms

                        def emit_ctx(lag):
                            pt, f0, diag, c = lag
                            lhsT_v = (
                                va_sb[:, p, c, 0:65]
                                if even
                                else va_sb[:, p, c, 64:192]
                            )
                            nc.tensor.matmul(
                                ctx_out[:, f0:512] if diag else ctx_out,
                                lhsT=lhsT_v,
                                rhs=pt[:, f0:512],
                                start=(c == 0),
                                stop=(c == nch - 1),
                            )

                        # chunks in pairs sharing a 2-bank PSUM tile: two
                        # score matmuls, then ONE exp over both halves
                        # (ACT's ~340ns fixed cost per instruction made it
                        # the pipeline limiter at one exp per chunk).
                        for cp in range(nch // 2):
                            c0 = 2 * cp
                            st = stp.tile([128, 1024], F32, tag="st", name="st")
                            pt = ptp.tile([128, 1024], BF16, tag="pt", name="pt")
                            diag = c0 >= qb * 4
                            # columns [0, f0) of a diag block are fully
                            # masked (q < kv everywhere): skip them.
                            f00 = 128 * (c0 - qb * 4) if diag else 0
                            for ci in range(2):
                                c = c0 + ci
                                f0 = f00 + 128 * ci if diag else 0
                                nc.tensor.matmul(
                                    st[:, ci * 512 + f0:(ci + 1) * 512],
                                    lhsT=kt_sb[j * 64:(j + 1) * 64, p,
                                               c * 128:(c + 1) * 128],
                                    rhs=qs[:, f0:512],
                                    start=True,
                                    stop=True,
                                )
                            # one exp across both banks; for diag pairs the
                            # staircase leaves a gap of unwritten columns in
                            # the middle - exp'd garbage no one ever reads.
                            nc.scalar.activation(
                                out=pt[:, f00:1024],
                                in_=st[:, f00:1024],
                                func=mybir.ActivationFunctionType.Exp,
                            )
                            if diag:
                                # zero the two masked 128-wide triangle
                                # blocks (at f00 and 640+f00) in one DVE op;
                                # cheaper than the PE -1e9 add and keeps the
                                # denominator (ones-column matmul) exact.
                                blk = pt[:, f00:f00 + 128]
                                dst = bass.AP(
                                    tensor=blk.tensor, offset=blk.offset,
                                    ap=[[blk.ap[0][0], 128], [640, 2], [1, 128]],
                                )
                                msk = bass.AP(
                                    tensor=mask_sb.tensor, offset=mask_sb.offset,
                                    ap=[[mask_sb.ap[0][0], 128], [0, 2], [1, 128]],
                                )
                                nc.vector.tensor_mul(
                                    out=dst, in0=dst, in1=msk
                                )
                            # ctx trails two pairs behind its exp so the PE
                            # queue stays deep (hides ACT latency, semaphore
                            # propagation, and LDWEIGHTS of the next matmul).
                            for ci in range(2):
                                f0 = f00 + 128 * ci if diag else 0
                                lagged.append(
                                    (pt[:, ci * 512:(ci + 1) * 512],
                                     f0, diag, c0 + ci)
                                )
                            while len(lagged) > 4:
                                emit_ctx(lagged.pop(0))
                        for lag in lagged:
                            emit_ctx(lag)

                        if pending is not None:
                            emit_norm(*pending)
                        pending = (ctx_ps, even, p, qb, h)
                        if h == 0 and qb > 0:
                            emit_outproj(qb - 1)

                emit_norm(*pending)
                pending = None
                emit_outproj(NS - 1)
    if not nc.is_finalized():
        nc.finalize()
    return nc


def _prep_inputs(embeddings, Wq, bq, Wk, bk, Wv, bv, Wo, bo):
    embeddings = np.asarray(embeddings, np.float32)
    Wq, bq = np.asarray(Wq, np.float32), np.asarray(bq, np.float32)
    Wk = np.asarray(Wk, np.float32)
    Wv, bv = np.asarray(Wv, np.float32), np.asarray(bv, np.float32)
    Wo = np.asarray(Wo, np.float32)

    import ml_dtypes
    bf16_t = ml_dtypes.bfloat16
    # mask01[p, j] = 1 where col j (query) >= partition p (key) in the
    # 128x128 diagonal block, else 0
    mask01 = np.triu(np.ones((128, 128), np.float32)).astype(bf16_t)
    vfix = np.zeros((128, 64), np.float32)
    vfix[:, 0] = 1.0
    vfix = vfix.astype(bf16_t)
    ones = np.ones((1, 128), np.float32).astype(bf16_t)

    in_maps = []
    for c in range(NCORES):
        b, g = c // 4, c % 4
        hs = HPC * g
        # [128, ND, S]: partition = d % 128, chunk = d // 128
        xt = np.ascontiguousarray(
            embeddings[b].T.reshape(ND, 128, S).transpose(1, 0, 2)
        ).astype(bf16_t)
        # 1/sqrt(dk) folded into Wq/bq (exact power of two)
        wq2 = np.stack(
            [np.concatenate([Wq[hs + 2 * p], Wq[hs + 2 * p + 1]], axis=1)
             for p in range(NPAIR)]
        ) * 0.125
        wk2 = np.stack(
            [np.concatenate([Wk[hs + 2 * p], Wk[hs + 2 * p + 1]], axis=1)
             for p in range(NPAIR)]
        )
        # [NPAIR, D, 128] -> [128, NPAIR, ND, 128] single-DMA layout
        wq2 = wq2.reshape(NPAIR, ND, 128, 128).transpose(2, 0, 1, 3)
        wk2 = wk2.reshape(NPAIR, ND, 128, 128).transpose(2, 0, 1, 3)
        wv4 = np.concatenate([Wv[hs + h] for h in range(HPC)], axis=1)
        wv4 = wv4.reshape(ND, 128, 256).transpose(1, 0, 2)
        wo4 = Wo[hs * DK:(hs + HPC) * DK, :].reshape(2, 128, D).transpose(1, 0, 2)
        bq2 = np.stack(
            [np.concatenate([bq[hs + 2 * p], bq[hs + 2 * p + 1]]) / 8.0
             for p in range(NPAIR)], axis=1
        )
        bvb = np.zeros((128, NPAIR, 2, 64), np.float32)
        for p in range(NPAIR):
            bvb[:, p, 0, :] = bv[hs + 2 * p][None, :]
            bvb[:, p, 1, :] = bv[hs + 2 * p + 1][None, :]
        in_maps.append({
            "xt": xt,
            "wq": np.ascontiguousarray(wq2).astype(bf16_t),
            "wk": np.ascontiguousarray(wk2).astype(bf16_t),
            "wv": np.ascontiguousarray(wv4).astype(bf16_t),
            "wo": np.ascontiguousarray(wo4).astype(bf16_t),
            "bq": np.ascontiguousarray(bq2),
            "bv_bc": bvb,
            "mask01": mask01,
            "ones": ones,
            "vfix": vfix,
        })
    return in_maps


def kernel(embeddings, Wq, bq, Wk, bk, Wv, bv, Wo, bo, _trace=False, _trace_kw=None):
    if "nc" not in _CACHE:
        _CACHE["nc"] = _build_bass()
    nc = _CACHE["nc"]
    in_maps = _prep_inputs(embeddings, Wq, bq, Wk, bk, Wv, bv, Wo, bo)
    kw = dict(_trace_kw or {})
    res = run_bass_kernel_spmd(
        nc, in_maps, core_ids=list(range(NCORES)), trace=_trace, **kw
    )
    _CACHE["last_result"] = res
    bo32 = np.asarray(bo, np.float32)
    out = np.empty((B, S, D), np.float32)
    for b in range(B):
        acc = np.asarray(res.results[4 * b]["out"], np.float32).copy()
        for g in range(1, 4):
            acc += np.asarray(res.results[4 * b + g]["out"], np.float32)
        out[b] = acc + bo32
    return out
